# revision 12
# baseline (speedup 1.0000x reference)
"""MoE (32 experts, top-4, T=64, H=2048, I=1408) — expert-parallel Bass kernel
for 8 trn2 NeuronCores.

Strategy (hardcoded, matches the expert-parallel sharding hint):
  - Each core owns 4 experts; the host ships weight shards pre-transposed and
    pre-tiled into the exact SBUF layout in bf16 so every DMA is a long
    contiguous run per partition.
  - x (as x.T, bf16) and router logits are replicated; logits columns are
    permuted per-core so the local experts are always columns 0..3 (keeps
    the SPMD program identical across cores).
  - Main loop streams weights at single-i-chunk granularity (44 groups of
    1.05 MB wgu + 0.53 MB wd): fine-grained delivery keeps both HWDGE rings
    and the SWDGE ring continuously busy, lets the first matmul start ~14 us
    in, and keeps the tensor engine warm (HAM at 2.4 GHz).  gate+up are
    interleaved in one DRAM tensor and delivered by a single DMA per chunk,
    alternating between the two HWDGE rings (SP/ACT); wd streams on the
    Pool ring (SWDGE); 8 chunks of issue lookahead.
  - The per-token routing weight G[:, e] is folded into the mix operand
    BEFORE the down projection (mix = silu(gate)*up*G broadcast over the
    i-partitions via a tiny PE broadcast matmul), so the down matmuls of
    ALL four experts accumulate into one [tok, H] PSUM group — no
    per-expert scale/accumulate pass, no cross-engine serialization at
    expert boundaries.
  - The final [tok, H] PSUM is copied to bf16 and exchanged with an
    AllToAll (single-hop mesh, cheaper than multi-stage ReduceScatter);
    each core tree-adds the 8 received partials for its 8-token shard on
    the DVE and writes the result (host casts back to f32).  A dummy 2 KB
    collective issued at kernel start absorbs the ncfw cold-start latency.
"""

import sys

sys.path.insert(0, "/opt/trn_rl_repo")

import ml_dtypes
import numpy as np

import concourse.bass as bass
import concourse.tile as tile
from concourse import bacc, bass_utils, mybir

T = 64
H = 2048
I = 1408
E = 32
TOPK = 4
NCORES = 8
EPC = E // NCORES  # experts per core
HC = H // 128  # 16 h-chunks
IC = I // 128  # 11 i-chunks
TSH = T // NCORES  # tokens per output shard

# weight-stream groups: 2 i-chunks per transfer (2.1 MB wgu + 1.05 MB wd —
# large enough for near-peak HWDGE throughput), 1-chunk tail per expert
# (IC=11 is odd); compute still proceeds chunk by chunk.
GROUPS = [
    (e, c0, min(2, IC - c0)) for e in range(EPC) for c0 in range(0, IC, 2)
]
# chunk idx -> (group, slot-in-group)
CHUNKS = [
    (gi, j) for gi, (e, c0, cn) in enumerate(GROUPS) for j in range(cn)
]
SCHED = [(e, c0 + j) for (e, c0, cn) in GROUPS for j in range(cn)]
LOOKAHEAD_G = 4  # groups of issue lookahead (~8 chunks)

f32 = mybir.dt.float32
bf16 = mybir.dt.bfloat16
Alu = mybir.AluOpType
Act = mybir.ActivationFunctionType

_BF16 = np.dtype(ml_dtypes.bfloat16)


def _build_program():
    nc = bacc.Bacc(
        "TRN2",
        target_bir_lowering=False,
        debug=False,
        enable_asserts=False,
        num_devices=NCORES,
    )

    xT_d = nc.dram_tensor("xT", [128, HC, T], bf16, kind="ExternalInput")
    lg_d = nc.dram_tensor("logits", [T, E], f32, kind="ExternalInput")
    # [T, :T] = identity(T); [T, T:] = ones(T, 128)
    aux_d = nc.dram_tensor("aux", [T, T + 128], f32, kind="ExternalInput")
    # gate and up interleaved per i-chunk: one DMA delivers both
    wgu_d = nc.dram_tensor(
        "wguT", [EPC, 128, IC, 2, HC * 128], bf16, kind="ExternalInput"
    )
    wdT_d = nc.dram_tensor("wdT", [EPC, 128, IC, H], bf16, kind="ExternalInput")
    out_d = nc.dram_tensor("out", [TSH, H], bf16, kind="ExternalOutput")

    with tile.TileContext(nc) as tc:
        _kernel_body(tc, xT_d, lg_d, aux_d, wgu_d, wdT_d, out_d)
    nc.compile()
    return nc


def _kernel_body(tc, xT_d, lg_d, aux_d, wgu_d, wdT_d, out_d):
    nc = tc.nc
    from contextlib import ExitStack

    ctx = ExitStack()
    with ctx:
        const = ctx.enter_context(tc.tile_pool(name="const", bufs=1))
        small = ctx.enter_context(tc.tile_pool(name="small", bufs=2))
        wgu_pool = ctx.enter_context(tc.tile_pool(name="wgu", bufs=LOOKAHEAD_G + 2))
        wd_pool = ctx.enter_context(tc.tile_pool(name="wd", bufs=LOOKAHEAD_G + 1))
        s_pool = ctx.enter_context(tc.tile_pool(name="silu", bufs=4))
        m_pool = ctx.enter_context(tc.tile_pool(name="mix", bufs=4))
        psg_pool = ctx.enter_context(tc.tile_pool(name="psg", bufs=2, space="PSUM"))
        psu_pool = ctx.enter_context(tc.tile_pool(name="psu", bufs=2, space="PSUM"))
        psd_pool = ctx.enter_context(tc.tile_pool(name="psd", bufs=1, space="PSUM"))
        dram = ctx.enter_context(tc.tile_pool(name="dram", bufs=1, space="DRAM"))

        # ---- ncfw warm-up: a tiny collective at kernel start absorbs the
        # collective firmware's cold trigger latency so the real AllToAll
        # at the end starts promptly. ----
        warm_in = dram.tile([128, 8], bf16)
        warm_out = dram.tile([128, 8], bf16)
        nc.gpsimd.collective_compute(
            "AllToAll",
            Alu.bypass,
            replica_groups=[list(range(NCORES))],
            ins=[warm_in.opt()],
            outs=[warm_out.opt()],
        )

        # ---- head: logits + aux + x (transposed, bf16) on the ACT queue ----
        lg = const.tile([T, E], f32)
        nc.scalar.dma_start(lg[:], lg_d.ap())
        aux = const.tile([T, T + 128], f32)  # [:, :T]=I(T), [:, T:]=ones
        nc.scalar.dma_start(aux[:], aux_d.ap())
        xt = const.tile([128, HC, T], bf16)  # x.T as [h_par, h_chunk, tok]
        nc.scalar.dma_start(xt[:], xT_d.ap())

        # ---- weight streams: both on the two HWDGE rings (SP / ACT) —
        # group g's wgu rides ring g%2 and its wd rides the other ring, so
        # each ring carries a balanced (2.1 + 1.05) MB per two groups.
        # (SWDGE's ~2 us software overhead per transfer can't sustain the
        # wd cadence at this granularity.)  4 groups of issue lookahead. ----
        def issue(g):
            e, c0, cn = GROUPS[g]
            wgu = wgu_pool.tile([128, 2, 2, HC * 128], bf16, tag="wgu")
            q = nc.sync if g % 2 == 0 else nc.scalar
            qo = nc.scalar if g % 2 == 0 else nc.sync
            if g == 0:
                # chunk-by-chunk, gate/up halves split on the first chunk,
                # so the very first matmuls start as early as possible
                q.dma_start(wgu[:, 0, 0:1, :], wgu_d.ap()[e, :, c0, 0:1, :])
                q.dma_start(wgu[:, 0, 1:2, :], wgu_d.ap()[e, :, c0, 1:2, :])
                q.dma_start(wgu[:, 1, :, :], wgu_d.ap()[e, :, c0 + 1, :, :])
            elif g == len(GROUPS) - 1:
                # final (1-chunk) group: gate half first, then up half
                # (same ring, FIFO) so the tail's gate matmuls overlap
                # the up half's delivery
                q.dma_start(wgu[:, 0, 0:1, :], wgu_d.ap()[e, :, c0, 0:1, :])
                q.dma_start(wgu[:, 0, 1:2, :], wgu_d.ap()[e, :, c0, 1:2, :])
            else:
                q.dma_start(
                    wgu[:, :cn, :, :], wgu_d.ap()[e, :, c0 : c0 + cn, :, :]
                )
            wdt = wd_pool.tile([128, 2, H], bf16, tag="wd")
            qo.dma_start(wdt[:, :cn, :], wdT_d.ap()[e, :, c0 : c0 + cn, :])
            return wgu, wdt

        inflight = [issue(g) for g in range(LOOKAHEAD_G)]

        # ---- routing: top-4 + softmax over selected logits (DVE + one ACT op) ----
        work = small.tile([T, E], f32)
        nc.vector.tensor_copy(work[:], lg[:])
        negm0 = const.tile([T, 1], f32)
        mlast = const.tile([T, 1], f32)
        for k in range(TOPK):
            m = small.tile([T, 1], f32, tag="mk")
            nc.vector.tensor_reduce(m[:], work[:], axis=mybir.AxisListType.X, op=Alu.max)
            if k == 0:
                nc.vector.tensor_scalar_mul(negm0[:], m[:], -1.0)
            if k == TOPK - 1:
                nc.vector.tensor_copy(mlast[:], m[:])
            else:
                eq = small.tile([T, E], f32, tag="eq")
                nc.vector.tensor_scalar(eq[:], work[:], m[:], None, op0=Alu.is_equal)
                nc.vector.tensor_scalar(eq[:], eq[:], 1e30, None, op0=Alu.mult)
                nc.vector.tensor_tensor(work[:], work[:], eq[:], op=Alu.subtract)

        sel = small.tile([T, E], f32)
        nc.vector.tensor_scalar(sel[:], lg[:], mlast[:], None, op0=Alu.is_ge)
        ex = small.tile([T, E], f32)
        nc.scalar.activation(ex[:], lg[:], func=Act.Exp, bias=negm0[:], scale=1.0)
        nc.vector.tensor_tensor(ex[:], ex[:], sel[:], op=Alu.mult)
        den = small.tile([T, 1], f32)
        nc.vector.reduce_sum(den[:], ex[:], axis=mybir.AxisListType.X)
        rec = small.tile([T, 1], f32)
        nc.vector.reciprocal(rec[:], den[:])
        G = const.tile([T, E], f32)  # routing weights, local experts = cols 0..EPC-1
        nc.vector.tensor_scalar(G[:], ex[:], rec[:], None, op0=Alu.mult)

        # ---- broadcast G[:, e] across the 128 i-partitions:
        # diag_e = I(T) * G[:, e] (per-partition scalar), then
        # Gb_e[p, t] = ones[k, p].T @ diag_e[k, t] = G[t, e]. ----
        Gb = []
        for e in range(EPC):
            diag = small.tile([T, T], f32, tag="diag")
            nc.vector.tensor_scalar(
                diag[:], aux[:, 0:T], G[:, e : e + 1], None, op0=Alu.mult
            )
            psb = psg_pool.tile([128, T], f32, tag="g")
            nc.tensor.matmul(
                psb[:], aux[:, T : T + 128], diag[:],
                start=True, stop=True, skip_group_check=True,
            )
            gb = const.tile([128, T], f32, tag=f"gb{e}")
            nc.vector.tensor_copy(gb[:], psb[:])
            Gb.append(gb)

        # ---- main per-i-chunk pipeline ----
        cc_stage = const.tile([T, H], bf16)
        cc_in = dram.tile([T, H], bf16)

        psd = psd_pool.tile([T, H], f32)

        def down(idx, mix_t, wdt, slot):
            # the down matmuls for chunk idx (issued one chunk late so the
            # PE never bubbles on the ACT->DVE silu/mix chain)
            for b in range(H // 512):
                nc.tensor.matmul(
                    psd[:, 512 * b : 512 * (b + 1)],
                    mix_t[:],
                    wdt[:, slot, 512 * b : 512 * (b + 1)],
                    start=(idx == 0),
                    stop=(idx == len(SCHED) - 1),
                    skip_group_check=True,
                )

        pend = None  # (idx, mix_t, wdt, slot) awaiting its down matmuls
        for idx, (e, c) in enumerate(SCHED):
            gi, j = CHUNKS[idx]
            wgu, wdt = inflight[gi]
            psg = psg_pool.tile([128, T], f32, tag="g")
            psu = psu_pool.tile([128, T], f32, tag="u")

            # gate block, then up block (weight-stationary, x moving)
            for gu, ps in ((0, psg), (1, psu)):
                for hc in range(HC):
                    nc.tensor.matmul(
                        ps[:],
                        wgu[:, j, gu, 128 * hc : 128 * (hc + 1)],
                        xt[:, hc, :],
                        start=(hc == 0),
                        stop=(hc == HC - 1),
                        skip_group_check=True,
                    )

            # keep the DMA queues fed before any ACT/DVE sem-waits queue up
            # (once per group, when its last chunk's gate/up are issued)
            if j == GROUPS[gi][2] - 1 and gi + LOOKAHEAD_G < len(GROUPS):
                inflight.append(issue(gi + LOOKAHEAD_G))

            if pend is not None:
                down(*pend)

            silu_t = s_pool.tile([128, T], f32, tag="silu")
            nc.scalar.activation(silu_t[:], psg[:], func=Act.Silu)
            tmp_t = s_pool.tile([128, T], f32, tag="tmp")
            nc.vector.tensor_tensor(tmp_t[:], silu_t[:], psu[:], op=Alu.mult)
            mix_t = m_pool.tile([128, T], bf16, tag="mix")
            nc.vector.tensor_tensor(mix_t[:], tmp_t[:], Gb[e][:], op=Alu.mult)
            pend = (idx, mix_t, wdt, j)
        down(*pend)

        # ---- stage the G-weighted sum (bf16) and exchange shards ----
        for b in range(H // 512):
            sl = slice(512 * b, 512 * (b + 1))
            nc.vector.tensor_copy(cc_stage[:, sl], psd[:, sl])
            q = nc.sync if b % 2 == 0 else nc.scalar
            q.dma_start(cc_in[:, sl], cc_stage[:, sl])

        # AllToAll: rank j receives every rank's partial for its own
        # 8-token shard, [8 ranks][TSH, H] stacked on the token axis.
        cc_a2a = dram.tile([T, H], bf16)
        nc.gpsimd.collective_compute(
            "AllToAll",
            Alu.bypass,
            replica_groups=[list(range(NCORES))],
            ins=[cc_in.opt()],
            outs=[cc_a2a.opt()],
        )

        # tree-reduce the 8 partials ([TSH, H] = 32 KB each, viewed as
        # [128, 128] for full-width DVE adds), then emit the shard
        red = []
        for j in range(NCORES):
            r = const.tile([128, 128], bf16, tag=f"red{j}")
            q = (nc.sync, nc.scalar, nc.gpsimd)[j % 3]
            q.dma_start(r[:], cc_a2a[TSH * j : TSH * (j + 1), :])
            red.append(r)
        for span in (1, 2, 4):
            for j in range(0, NCORES, 2 * span):
                nc.vector.tensor_tensor(
                    red[j][:], red[j][:], red[j + span][:], op=Alu.add
                )
        nc.sync.dma_start(out_d.ap(), red[0][:])


_PROGRAM = None


def _get_program():
    global _PROGRAM
    if _PROGRAM is None:
        _PROGRAM = _build_program()
    return _PROGRAM


def _gateup_layout(wg, wu):
    """gate, up [n, I, H] -> [n, 128, IC, 2, HC*128] bf16 interleaved:
    stationary tiles are [h-in-chunk (partitions), i-in-chunk]; per
    (expert, partition, i-chunk) the gate and up 4 KB runs are adjacent,
    so one DMA per chunk delivers both (hc-major within each)."""
    n = wg.shape[0]

    def lay(w):
        b = w.reshape(n, IC, 128, HC, 128)  # [e, ic, q(i), hc, p(h)]
        return b.transpose(0, 4, 1, 3, 2).reshape(n, 128, IC, HC * 128)

    v = np.stack([lay(wg), lay(wu)], axis=3)  # [n, 128, IC, 2, HC*128]
    return np.ascontiguousarray(v.astype(_BF16))


def _down_layout(w):
    """[n, H, I] -> [n, 128, IC, H] bf16: moving rows are [i-in-chunk
    (partitions), h]; per-(expert, partition, i-chunk) contiguous 4 KB."""
    n = w.shape[0]
    a = w.transpose(0, 2, 1).reshape(n, IC, 128, H).transpose(0, 2, 1, 3)
    return np.ascontiguousarray(a.astype(_BF16))


def _make_in_maps(x, router_logits, w_gate, w_up, w_down):
    xT = np.ascontiguousarray(
        np.asarray(x, np.float32).T.reshape(HC, 128, T).transpose(1, 0, 2).astype(_BF16)
    )
    aux = np.concatenate(
        [np.eye(T, dtype=np.float32), np.ones((T, 128), dtype=np.float32)], axis=1
    )
    in_maps = []
    for c in range(NCORES):
        lo, hi = c * EPC, (c + 1) * EPC
        perm = list(range(lo, hi)) + [i for i in range(E) if not (lo <= i < hi)]
        lg_c = np.ascontiguousarray(router_logits[:, perm].astype(np.float32, copy=False))
        in_maps.append(
            {
                "xT": xT,
                "logits": lg_c,
                "aux": aux,
                "wguT": _gateup_layout(w_gate[lo:hi], w_up[lo:hi]),
                "wdT": _down_layout(w_down[lo:hi]),
            }
        )
    return in_maps


def kernel(x, router_logits, w_gate, w_up, w_down, _trace=False, _results_out=None):
    x = np.asarray(x, dtype=np.float32)
    router_logits = np.asarray(router_logits, dtype=np.float32)
    w_gate = np.asarray(w_gate, dtype=np.float32)
    w_up = np.asarray(w_up, dtype=np.float32)
    w_down = np.asarray(w_down, dtype=np.float32)

    nc = _get_program()
    in_maps = _make_in_maps(x, router_logits, w_gate, w_up, w_down)
    res = bass_utils.run_bass_kernel_spmd(
        nc, in_maps, core_ids=list(range(NCORES)), trace=_trace
    )
    if _results_out is not None:
        _results_out.append(res)
    shards = [res.results[c]["out"] for c in range(NCORES)]
    out = np.concatenate(shards, axis=0)  # [T, H] bf16
    return out[:, None, :].astype(np.float32)
